# revision 5
# baseline (speedup 1.0000x reference)
"""DEVISE margin hinge loss on 8 Trainium2 NeuronCores (Bass/Tile).

Data-parallel: batch sharded 8 ways, weights + label embeddings replicated.
Per core: one matmul chain produces projT duplicated into both PSUM halves
(W columns pre-duplicated at pack time); true-sim via elementwise mult +
ones-matmul partition reduction; hinge sweep tiles sims into (128,1024)
PSUM slots consumed by ACT (relu+bias+accum fused, 11 slots per m-chunk)
and DVE (scalar_tensor_tensor+accum, 9 slots) to match their 1.2 : 0.96
GHz rates. Loads spread over 4 engine HWDGE queues so DMA overlaps the
sweep. Partial scalar per core; host sums and normalizes.
"""

import numpy as np

B, D, C, DC = 4096, 1024, 20000, 64
MARGIN = 0.1
NCORES = 8
BL = B // NCORES           # 512 local batch
M_CHUNKS = BL // 128       # 4
K_CHUNKS = D // 128        # 8
CP_LO = 10240              # classes in partitions 0:64 of et
CP_HI = C - CP_LO          # 9760 classes in partitions 64:128
ET_TILES = 5               # (128, 2048) SBUF tiles covering et
G_PER_M = 10               # 1024-wide pair groups per m-chunk sweep
HI_LAST = CP_HI - 9 * 1024  # 544: width of the last hi slot
ACT_HI_G = 5               # the hi group routed to ACT (11:9 balance)
N_A = M_CHUNKS * (G_PER_M + 1)   # 44 ACT slots
N_D = M_CHUNKS * (G_PER_M - 1)   # 36 DVE slots

_cache = {}


def _build_nc(reps: int = 1, variant: str = "full"):
    import concourse.bacc as bacc
    import concourse.mybir as mybir
    import concourse.tile as tile

    dt = mybir.dt.float32
    bf = mybir.dt.bfloat16
    Act = mybir.ActivationFunctionType
    Alu = mybir.AluOpType

    nc = bacc.Bacc()
    xt_d = nc.declare_dram_parameter("xt", [128, K_CHUNKS * BL], bf, isOutput=False)
    w2_d = nc.declare_dram_parameter("w2", [128, K_CHUNKS * 128], bf, isOutput=False)
    et_d = nc.declare_dram_parameter("et", [128, ET_TILES * 2048], bf, isOutput=False)
    eyt_d = nc.declare_dram_parameter("eyt", [128, BL], dt, isOutput=False)
    out_d = nc.declare_dram_parameter("out", [1, 1], dt, isOutput=True)

    with tile.TileContext(nc) as tc:
        def body(_iv=None):
            with tc.tile_pool(name="const", bufs=1) as cpool:
                # ---- loads spread over 4 HWDGE queues ---------------------
                w2_sb = cpool.tile([128, K_CHUNKS * 128], bf, tag="w2")
                nc.scalar.dma_start(w2_sb[:], w2_d[:])
                et_sb = []
                for i in range(ET_TILES):
                    t = cpool.tile([128, 2048], bf, tag=f"et{i}")
                    et_sb.append(t)
                nc.scalar.dma_start(et_sb[0][:], et_d[:, 0:2048])
                xt_sb = cpool.tile([128, K_CHUNKS * BL], bf, tag="xt")
                for h in range(2):
                    nc.sync.dma_start(
                        xt_sb[:, h * 4 * BL : (h + 1) * 4 * BL],
                        xt_d[:, h * 4 * BL : (h + 1) * 4 * BL],
                    )
                eyt_sb = cpool.tile([128, BL], dt, tag="eyt")
                nc.scalar.dma_start(eyt_sb[:], eyt_d[:])
                nc.sync.dma_start(et_sb[1][:], et_d[:, 2048:4096])
                nc.scalar.dma_start(et_sb[2][:], et_d[:, 4096:6144])
                nc.sync.dma_start(et_sb[3][:], et_d[:, 6144:8192])
                nc.gpsimd.dma_start(et_sb[4][:], et_d[:, 8192:10240])

                projT = cpool.tile([128, BL], bf, tag="projT")
                tmul = cpool.tile([128, BL], dt, tag="tmul")
                bias_col = cpool.tile([128, M_CHUNKS], dt, tag="bias")
                ones_col = cpool.tile([128, 1], dt, tag="ones")
                nc.vector.memset(ones_col[:], 1.0)
                stats_a = cpool.tile([128, N_A], dt, tag="stats_a")
                stats_d = cpool.tile([128, N_D], dt, tag="stats_d")
                nc.gpsimd.memset(stats_a[:], 0.0)
                nc.vector.memset(stats_d[:], 0.0)
                zeros = cpool.tile([128, 1024], dt, tag="zeros")
                nc.vector.memset(zeros[:], 0.0)
                # single-buffer scratch, each written by exactly one engine
                a_scr = cpool.tile([128, 1024], dt, tag="ascr")
                d_scr = cpool.tile([128, 1024], dt, tag="dscr")

                if variant == "dma":
                    with tc.tile_pool(name="pdma", bufs=1, space="PSUM") as pd:
                        total_s = cpool.tile([1, 1], dt, tag="totscalar")
                        for t in [*et_sb, xt_sb, w2_sb, eyt_sb]:
                            tt = pd.tile([1, 1], dt, tag="touch")
                            nc.tensor.matmul(
                                tt[:], t[:, 0:1], t[:, 0:1], start=True, stop=True
                            )
                        nc.vector.memset(total_s[:], 0.0)
                        nc.sync.dma_start(out_d[:], total_s[:])
                    return

                # ---- phase 1: projT (both halves in one chain) + bias -----
                with tc.tile_pool(name="ppre", bufs=1, space="PSUM") as ppre:
                    psum_pr = ppre.tile([128, BL], dt, tag="pp")
                    for k in range(K_CHUNKS):
                        nc.tensor.matmul(
                            psum_pr[:],
                            w2_sb[:, k * 128 : (k + 1) * 128],
                            xt_sb[:, k * BL : (k + 1) * BL],
                            start=(k == 0),
                            stop=(k == K_CHUNKS - 1),
                        )
                    nc.scalar.copy(projT[:], psum_pr[:])
                    nc.vector.tensor_mul(tmul[:], psum_pr[:], eyt_sb[:])
                    psum_t = ppre.tile([128, M_CHUNKS], dt, tag="pt")
                    for m in range(M_CHUNKS):
                        nc.tensor.matmul(
                            psum_t[:, m : m + 1],
                            tmul[:, m * 128 : (m + 1) * 128],
                            ones_col[:],
                            start=True,
                            stop=True,
                        )
                    # bias = margin - t, where t = 0.5 * (both-halves sum)
                    nc.vector.tensor_scalar(
                        bias_col[:], psum_t[:], -0.5, MARGIN,
                        op0=Alu.mult, op1=Alu.add,
                    )

                # ---- phase 2: hinge sweep ---------------------------------
                it_a = 0
                it_d = 0
                with (
                    tc.tile_pool(name="pa", bufs=2, space="PSUM") as pa,
                    tc.tile_pool(name="pd", bufs=2, space="PSUM") as pd2,
                ):
                    for m in range(M_CHUNKS):
                        bias_m = bias_col[:, m : m + 1]
                        lhs_lo = projT[0:64, m * 128 : (m + 1) * 128]
                        lhs_hi = projT[64:128, m * 128 : (m + 1) * 128]
                        for g in range(G_PER_M):
                            eti, off = divmod(g, 2)
                            cs = off * 1024
                            # lo half -> ACT (512-col matmuls: one PSUM bank each)
                            slot_a = pa.tile([128, 1024], dt, tag="pa")
                            for s in range(2):
                                nc.tensor.matmul(
                                    slot_a[:, s * 512 : (s + 1) * 512],
                                    lhs_lo,
                                    et_sb[eti][0:64, cs + s * 512 : cs + (s + 1) * 512],
                                    start=True,
                                    stop=True,
                                    tile_position=(0, 0),
                                )
                            if variant != "nocons":
                                nc.scalar.activation(
                                    a_scr[:], slot_a[:], Act.Relu,
                                    bias=bias_m, scale=1.0,
                                    accum_out=stats_a[:, it_a : it_a + 1],
                                )
                            it_a += 1
                            # hi half -> DVE (except one group to ACT)
                            w = HI_LAST if g == G_PER_M - 1 else 1024
                            to_act = g == ACT_HI_G
                            pool = pa if to_act else pd2
                            slot_h = pool.tile([128, 1024], dt, tag="pa" if to_act else "pd")
                            for s0 in range(0, w, 512):
                                sw = min(512, w - s0)
                                nc.tensor.matmul(
                                    slot_h[:, s0 : s0 + sw],
                                    lhs_hi,
                                    et_sb[eti][64:128, cs + s0 : cs + s0 + sw],
                                    start=True,
                                    stop=True,
                                    tile_position=(64, 0),
                                )
                            if variant == "nocons":
                                continue
                            if to_act:
                                nc.scalar.activation(
                                    a_scr[:, 0:w], slot_h[:, 0:w], Act.Relu,
                                    bias=bias_m, scale=1.0,
                                    accum_out=stats_a[:, it_a : it_a + 1],
                                )
                                it_a += 1
                            else:
                                nc.vector.scalar_tensor_tensor(
                                    out=d_scr[:, 0:w],
                                    in0=slot_h[:, 0:w],
                                    scalar=bias_m,
                                    in1=zeros[:, 0:w],
                                    op0=Alu.add,
                                    op1=Alu.max,
                                    accum_out=stats_d[:, it_d : it_d + 1],
                                )
                                it_d += 1

                    # ---- phase 3: final scalar ----------------------------
                    red_a = cpool.tile([128, 1], dt, tag="red_a")
                    nc.vector.tensor_reduce(
                        red_a[:], stats_a[:], axis=mybir.AxisListType.X, op=Alu.add
                    )
                    red_d = cpool.tile([128, 1], dt, tag="red_d")
                    nc.vector.tensor_reduce(
                        red_d[:], stats_d[:], axis=mybir.AxisListType.X, op=Alu.add
                    )
                    total_col = cpool.tile([128, 1], dt, tag="total")
                    nc.vector.tensor_add(total_col[:], red_a[:], red_d[:])
                    total_s = cpool.tile([1, 1], dt, tag="totscalar")
                    fin_slot = pa.tile([128, 1024], dt, tag="pa")
                    # touch absorbs the DVE wait for total_col, then the real
                    # 1x1 matmul sums total_col over partitions via ones
                    nc.tensor.matmul(
                        fin_slot[0:1, 0:1], total_col[:], total_col[:],
                        start=True, stop=True,
                    )
                    nc.tensor.matmul(
                        fin_slot[0:1, 0:1], total_col[:], ones_col[:],
                        start=True, stop=True,
                    )
                    nc.vector.tensor_copy(total_s[:], fin_slot[0:1, 0:1])
                nc.sync.dma_start(out_d[:], total_s[:])

        if reps == 1:
            body()
        else:
            with tc.For_i(0, reps, 1) as iv:
                body(iv)

    nc.finalize()
    return nc


def _pack_inputs(X, y, E, W):
    """Per-core DRAM images. Layouts match the device program above."""
    import ml_dtypes

    bf16 = ml_dtypes.bfloat16
    X = np.ascontiguousarray(np.asarray(X, dtype=np.float32))
    y = np.asarray(y).astype(np.int64)
    E = np.ascontiguousarray(np.asarray(E, dtype=np.float32))
    W = np.ascontiguousarray(np.asarray(W, dtype=np.float32))

    # w2: per k-chunk, W[k] duplicated along columns -> [128, 8*128]
    w2_pack = np.concatenate(
        [np.concatenate([W[k * 128 : (k + 1) * 128]] * 2, axis=1) for k in range(K_CHUNKS)],
        axis=1,
    ).astype(bf16)
    w2_pack = np.ascontiguousarray(w2_pack)
    Et = E.T  # (64, C)
    et_pack = np.zeros((128, CP_LO), dtype=np.float32)
    et_pack[:64, :] = Et[:, :CP_LO]
    et_pack[64:, :CP_HI] = Et[:, CP_LO:]
    et_pack = np.ascontiguousarray(et_pack.astype(bf16))

    in_maps = []
    for s in range(NCORES):
        Xs = X[s * BL : (s + 1) * BL]  # (BL, D)
        xt_pack = np.ascontiguousarray(
            Xs.T.reshape(K_CHUNKS, 128, BL).transpose(1, 0, 2).reshape(128, K_CHUNKS * BL)
        ).astype(bf16)
        EyT = E[y[s * BL : (s + 1) * BL]].T  # (DC, BL)
        eyt_pack = np.ascontiguousarray(np.concatenate([EyT, EyT], axis=0))
        in_maps.append({"xt": xt_pack, "w2": w2_pack, "et": et_pack, "eyt": eyt_pack})
    return in_maps


def run_spmd(in_maps, reps: int = 1, trace: bool = False):
    from concourse.bass_utils import run_bass_kernel_spmd

    key = reps
    if key not in _cache:
        _cache[key] = _build_nc(reps)  # full variant only
    nc = _cache[key]
    return run_bass_kernel_spmd(
        nc, in_maps, core_ids=list(range(NCORES)), trace=trace
    )


def kernel(X, y, label_embeddings, weights):
    in_maps = _pack_inputs(X, y, label_embeddings, weights)
    res = run_spmd(in_maps).results
    total = sum(float(res[s]["out"][0, 0]) for s in range(NCORES))
    loss = np.float32(total / B - MARGIN)
    return np.array([loss], dtype=np.float32)


# revision 7
# speedup vs baseline: 1.2513x; 1.2513x over previous
"""DEVISE margin hinge loss on 8 Trainium2 NeuronCores (Bass/Tile).

Data-parallel: batch sharded 8 ways, weights + label embeddings replicated.
Per core: one matmul chain produces projT duplicated into both PSUM halves
(W columns pre-duplicated at pack time); true-sim via elementwise mult +
ones-matmul partition reduction; hinge sweep tiles sims into (128,1024)
PSUM slots consumed by ACT (relu+bias+accum fused, 11 slots per m-chunk)
and DVE (scalar_tensor_tensor+accum, 9 slots) to match their 1.2 : 0.96
GHz rates. Loads spread over 4 engine HWDGE queues so DMA overlaps the
sweep. Partial scalar per core; host sums and normalizes.
"""

import numpy as np

B, D, C, DC = 4096, 1024, 20000, 64
MARGIN = 0.1
NCORES = 8
BL = B // NCORES           # 512 local batch
M_CHUNKS = BL // 128       # 4
K_CHUNKS = D // 128        # 8
CP_LO = 10240              # classes in partitions 0:64 of et
CP_HI = C - CP_LO          # 9760 classes in partitions 64:128
ET_TILES = 5               # (128, 2048) SBUF tiles covering et
G_PER_M = 10               # 1024-wide pair groups per m-chunk sweep
HI_LAST = CP_HI - 9 * 1024  # 544: width of the last hi slot
ACT_HI_G = -1              # no hi group to ACT: ACT/DVE per-slot costs are ~equal
N_A = M_CHUNKS * G_PER_M   # 40 ACT slots
N_D = M_CHUNKS * G_PER_M   # 40 DVE slots

_cache = {}


def _build_nc(reps: int = 1, variant: str = "full"):
    import concourse.bacc as bacc
    import concourse.mybir as mybir
    import concourse.tile as tile

    dt = mybir.dt.float32
    bf = mybir.dt.bfloat16
    Act = mybir.ActivationFunctionType
    Alu = mybir.AluOpType

    nc = bacc.Bacc()
    xt_d = nc.declare_dram_parameter("xt", [128, K_CHUNKS * BL], bf, isOutput=False)
    w2_d = nc.declare_dram_parameter("w2", [128, K_CHUNKS * 128], bf, isOutput=False)
    et_d = nc.declare_dram_parameter("et", [128, ET_TILES * 2048], bf, isOutput=False)
    eyt_d = nc.declare_dram_parameter("eyt", [128, BL], dt, isOutput=False)
    out_d = nc.declare_dram_parameter("out", [1, 1], dt, isOutput=True)

    with tile.TileContext(nc) as tc:
        def body(_iv=None):
            with tc.tile_pool(name="const", bufs=1) as cpool:
                # ---- loads spread over 4 HWDGE queues ---------------------
                w2_sb = cpool.tile([128, K_CHUNKS * 128], bf, tag="w2")
                et_sb = []
                for i in range(ET_TILES):
                    t = cpool.tile([128, 2048], bf, tag=f"et{i}")
                    et_sb.append(t)
                nc.scalar.dma_start(et_sb[0][:], et_d[:, 0:2048])
                nc.scalar.dma_start(w2_sb[:], w2_d[:])
                xt_sb = cpool.tile([128, K_CHUNKS * BL], bf, tag="xt")
                for h in range(2):
                    nc.sync.dma_start(
                        xt_sb[:, h * 4 * BL : (h + 1) * 4 * BL],
                        xt_d[:, h * 4 * BL : (h + 1) * 4 * BL],
                    )
                eyt_sb = cpool.tile([128, BL], dt, tag="eyt")
                nc.scalar.dma_start(eyt_sb[:], eyt_d[:])
                nc.sync.dma_start(et_sb[1][:], et_d[:, 2048:4096])
                nc.scalar.dma_start(et_sb[2][:], et_d[:, 4096:6144])
                nc.sync.dma_start(et_sb[3][:], et_d[:, 6144:8192])
                nc.gpsimd.dma_start(et_sb[4][:], et_d[:, 8192:10240])

                projT = cpool.tile([128, BL], bf, tag="projT")
                tmul = cpool.tile([128, BL], dt, tag="tmul")
                bias_col = cpool.tile([128, M_CHUNKS], dt, tag="bias")
                ones_col = cpool.tile([128, 1], dt, tag="ones")
                nc.vector.memset(ones_col[:], 1.0)
                stats_a = cpool.tile([128, N_A], dt, tag="stats_a")
                stats_d = cpool.tile([128, N_D], dt, tag="stats_d")
                nc.gpsimd.memset(stats_a[:], 0.0)
                nc.vector.memset(stats_d[:], 0.0)
                zeros = cpool.tile([128, 1024], dt, tag="zeros")
                nc.vector.memset(zeros[:], 0.0)
                # single-buffer scratch, each written by exactly one engine
                a_scr = cpool.tile([128, 1024], dt, tag="ascr")
                d_scr = cpool.tile([128, 1024], dt, tag="dscr")

                if variant == "dma":
                    with tc.tile_pool(name="pdma", bufs=1, space="PSUM") as pd:
                        total_s = cpool.tile([1, 1], dt, tag="totscalar")
                        for t in [*et_sb, xt_sb, w2_sb, eyt_sb]:
                            tt = pd.tile([1, 1], dt, tag="touch")
                            nc.tensor.matmul(
                                tt[:], t[:, 0:1], t[:, 0:1], start=True, stop=True
                            )
                        nc.vector.memset(total_s[:], 0.0)
                        nc.sync.dma_start(out_d[:], total_s[:])
                    return

                # ---- phase 1: projT (both halves in one chain) + bias -----
                with tc.tile_pool(name="ppre", bufs=1, space="PSUM") as ppre:
                    psum_pr = ppre.tile([128, BL], dt, tag="pp")
                    for k in range(K_CHUNKS):
                        nc.tensor.matmul(
                            psum_pr[:],
                            w2_sb[:, k * 128 : (k + 1) * 128],
                            xt_sb[:, k * BL : (k + 1) * BL],
                            start=(k == 0),
                            stop=(k == K_CHUNKS - 1),
                        )
                    nc.scalar.copy(projT[:], psum_pr[:])
                    nc.vector.tensor_mul(tmul[:], psum_pr[:], eyt_sb[:])
                    psum_t = ppre.tile([128, M_CHUNKS], dt, tag="pt")
                    for m in range(M_CHUNKS):
                        nc.tensor.matmul(
                            psum_t[:, m : m + 1],
                            tmul[:, m * 128 : (m + 1) * 128],
                            ones_col[:],
                            start=True,
                            stop=True,
                        )
                    # bias = margin - t, where t = 0.5 * (both-halves sum)
                    nc.vector.tensor_scalar(
                        bias_col[:], psum_t[:], -0.5, MARGIN,
                        op0=Alu.mult, op1=Alu.add,
                    )

                # ---- phase 2: hinge sweep ---------------------------------
                it_a = 0
                it_d = 0
                with (
                    tc.tile_pool(name="pa", bufs=2, space="PSUM") as pa,
                    tc.tile_pool(name="pd", bufs=2, space="PSUM") as pd2,
                ):
                    for m in range(M_CHUNKS):
                        bias_m = bias_col[:, m : m + 1]
                        lhs_lo = projT[0:64, m * 128 : (m + 1) * 128]
                        lhs_hi = projT[64:128, m * 128 : (m + 1) * 128]
                        for g in range(G_PER_M):
                            eti, off = divmod(g, 2)
                            cs = off * 1024
                            # lo half -> ACT (512-col matmuls: one PSUM bank each)
                            slot_a = pa.tile([128, 1024], dt, tag="pa")
                            for s in range(2):
                                nc.tensor.matmul(
                                    slot_a[:, s * 512 : (s + 1) * 512],
                                    lhs_lo,
                                    et_sb[eti][0:64, cs + s * 512 : cs + (s + 1) * 512],
                                    start=True,
                                    stop=True,
                                    tile_position=(0, 0),
                                )
                            if variant != "nocons":
                                nc.scalar.activation(
                                    a_scr[:], slot_a[:], Act.Relu,
                                    bias=bias_m, scale=1.0,
                                    accum_out=stats_a[:, it_a : it_a + 1],
                                )
                            it_a += 1
                            # hi half -> DVE (except one group to ACT)
                            w = HI_LAST if g == G_PER_M - 1 else 1024
                            to_act = g == ACT_HI_G
                            pool = pa if to_act else pd2
                            slot_h = pool.tile([128, 1024], dt, tag="pa" if to_act else "pd")
                            for s0 in range(0, w, 512):
                                sw = min(512, w - s0)
                                nc.tensor.matmul(
                                    slot_h[:, s0 : s0 + sw],
                                    lhs_hi,
                                    et_sb[eti][64:128, cs + s0 : cs + s0 + sw],
                                    start=True,
                                    stop=True,
                                    tile_position=(64, 0),
                                )
                            if variant == "nocons":
                                continue
                            if to_act:
                                nc.scalar.activation(
                                    a_scr[:, 0:w], slot_h[:, 0:w], Act.Relu,
                                    bias=bias_m, scale=1.0,
                                    accum_out=stats_a[:, it_a : it_a + 1],
                                )
                                it_a += 1
                            else:
                                nc.vector.scalar_tensor_tensor(
                                    out=d_scr[:, 0:w],
                                    in0=slot_h[:, 0:w],
                                    scalar=bias_m,
                                    in1=zeros[:, 0:w],
                                    op0=Alu.add,
                                    op1=Alu.max,
                                    accum_out=stats_d[:, it_d : it_d + 1],
                                )
                                it_d += 1

                    # ---- phase 3: final scalar ----------------------------
                    red_a = cpool.tile([128, 1], dt, tag="red_a")
                    nc.vector.tensor_reduce(
                        red_a[:], stats_a[:], axis=mybir.AxisListType.X, op=Alu.add
                    )
                    red_d = cpool.tile([128, 1], dt, tag="red_d")
                    nc.vector.tensor_reduce(
                        red_d[:], stats_d[:], axis=mybir.AxisListType.X, op=Alu.add
                    )
                    total_col = cpool.tile([128, 1], dt, tag="total")
                    nc.vector.tensor_add(total_col[:], red_a[:], red_d[:])
                    total_s = cpool.tile([1, 1], dt, tag="totscalar")
                    fin_slot = pa.tile([128, 1024], dt, tag="pa")
                    # touch absorbs the DVE wait for total_col, then the real
                    # 1x1 matmul sums total_col over partitions via ones
                    nc.tensor.matmul(
                        fin_slot[0:1, 0:1], total_col[:], total_col[:],
                        start=True, stop=True,
                    )
                    nc.tensor.matmul(
                        fin_slot[0:1, 0:1], total_col[:], ones_col[:],
                        start=True, stop=True,
                    )
                    nc.vector.tensor_copy(total_s[:], fin_slot[0:1, 0:1])
                nc.sync.dma_start(out_d[:], total_s[:])

        if reps == 1:
            body()
        else:
            with tc.For_i(0, reps, 1) as iv:
                body(iv)

    nc.finalize()
    return nc


def _pack_inputs(X, y, E, W):
    """Per-core DRAM images. Layouts match the device program above."""
    import ml_dtypes

    bf16 = ml_dtypes.bfloat16
    X = np.ascontiguousarray(np.asarray(X, dtype=np.float32))
    y = np.asarray(y).astype(np.int64)
    E = np.ascontiguousarray(np.asarray(E, dtype=np.float32))
    W = np.ascontiguousarray(np.asarray(W, dtype=np.float32))

    # w2: per k-chunk, W[k] duplicated along columns -> [128, 8*128]
    w2_pack = np.concatenate(
        [np.concatenate([W[k * 128 : (k + 1) * 128]] * 2, axis=1) for k in range(K_CHUNKS)],
        axis=1,
    ).astype(bf16)
    w2_pack = np.ascontiguousarray(w2_pack)
    Et = E.T  # (64, C)
    et_pack = np.zeros((128, CP_LO), dtype=np.float32)
    et_pack[:64, :] = Et[:, :CP_LO]
    et_pack[64:, :CP_HI] = Et[:, CP_LO:]
    et_pack = np.ascontiguousarray(et_pack.astype(bf16))

    in_maps = []
    for s in range(NCORES):
        Xs = X[s * BL : (s + 1) * BL]  # (BL, D)
        xt_pack = np.ascontiguousarray(
            Xs.T.reshape(K_CHUNKS, 128, BL).transpose(1, 0, 2).reshape(128, K_CHUNKS * BL)
        ).astype(bf16)
        EyT = E[y[s * BL : (s + 1) * BL]].T  # (DC, BL)
        eyt_pack = np.ascontiguousarray(np.concatenate([EyT, EyT], axis=0))
        in_maps.append({"xt": xt_pack, "w2": w2_pack, "et": et_pack, "eyt": eyt_pack})
    return in_maps


def run_spmd(in_maps, reps: int = 1, trace: bool = False):
    from concourse.bass_utils import run_bass_kernel_spmd

    key = reps
    if key not in _cache:
        _cache[key] = _build_nc(reps)  # full variant only
    nc = _cache[key]
    return run_bass_kernel_spmd(
        nc, in_maps, core_ids=list(range(NCORES)), trace=trace
    )


def kernel(X, y, label_embeddings, weights):
    in_maps = _pack_inputs(X, y, label_embeddings, weights)
    res = run_spmd(in_maps).results
    total = sum(float(res[s]["out"][0, 0]) for s in range(NCORES))
    loss = np.float32(total / B - MARGIN)
    return np.array([loss], dtype=np.float32)
